# revision 2
# baseline (speedup 1.0000x reference)
"""Correlation cost-volume kernel (max_displacement=4) for 8 Trainium2 cores.

Problem: in1, in2: [B=8, C=256, H=128, W=128] f32.
out[b, dy*9+dx, h, w] = sum_c in1[b,c,h,w] * pad(in2)[b, c, h+dy, w+dx]
(pad = 4 zeros on each spatial side), output [8, 81, 128, 128] f32.

Strategy (data-parallel, one batch sample per core):
  2D-tiled gram with tall tiles + windowed output.  Each matmul tile covers
  a 16h x 8w block of in1 pixels (M = 128 PSUM partitions) against its
  24 x 16 halo of padded in2 (N = 384 columns, C contracted as two K=128
  tiles in PSUM).  Gram column n = rh*16 + rw; the 81-entry band for pixel
  (mh, mw) lives at n = (mh+dy)*16 + (mw+dx), i.e. inside the contiguous
  column window [16*mh, 16*mh+144).  Pixels are mapped to partitions as
  m = mh*8 + mw, so the 16 partitions of a pixel-row *pair* mh in {2j,2j+1}
  share the window [32j, 32j+160) -- a plain rectangular (partition-range x
  column-range) slice that ONE standard 3-dim DMA can ship per pair.  The
  device therefore writes only 160 of the 384 gram columns per pixel
  (5.24 MB vs 12.6 MB for the full gram) in 8 big window DMAs, and the host
  finishes the (fine, per-pixel mw+dx) band gather with numpy for free.

  To keep HBM-write descriptors >=512 B (sub-512B writes degrade to RMW),
  the gram is stored in SBUF block-interleaved as [part][12 col-blocks of
  32][tile][32 cols]: a window is then 5 whole blocks and each write DMA
  moves 8 KB-contiguous runs.  in2 ships fully padded (no on-chip memsets,
  keeping gpsimd -- and its DRAIN -- off the critical path so the read
  stream starts right after the entry barrier).
"""

import ml_dtypes
import numpy as np

import concourse.bass as bass
import concourse.bacc as bacc
import concourse.mybir as mybir
from concourse.bass_utils import run_bass_kernel_spmd
from concourse.tile import TileContext

B, C, H, W = 8, 256, 128, 128
D = 4
ND = 2 * D + 1  # 9 displacements per axis
HP = H + 2 * D  # 136 padded rows
WP = W + 2 * D  # 136 padded cols
KT = C // 128  # 2 contraction tiles
TH, TW = 16, 8  # matmul tile = 16h x 8w pixels (128 = PSUM partition dim)
NHT, NWT = H // TH, W // TW  # 8 row-stripes x 16 tiles each
RH, RW = TH + 2 * D, TW + 2 * D  # 24 x 16 halo region
NR = RH * RW  # 384 gram columns per tile
NT = NHT * NWT  # 128 tiles
BW = 32  # gram col-block width (block-interleaved SBUF layout)
NBLK = NR // BW  # 12 col-blocks
NG = 8  # write groups (pixel-row pairs)
WINB = 5  # 5 col-blocks = 160-col window per group

OUT_DT = mybir.dt.bfloat16
_OUT_NP = ml_dtypes.bfloat16

_CACHED_NC = None


def _build_nc():
    bf16 = mybir.dt.bfloat16

    nc = bacc.Bacc()
    # in1 as [c][ht][kt][wt][m=mh*8+mw]; in2 fully zero-padded host-side as
    # [c][kt][hp][wp] so no engine has to memset before the loads start
    in1_t = nc.declare_dram_parameter("in1_t", [128, NHT, KT, NWT, 128], bf16, isOutput=False)
    in2_p = nc.declare_dram_parameter("in2_p", [128, KT, HP, WP], bf16, isOutput=False)
    # windowed gram: group j ships partitions [16j,16j+16) x col-blocks
    # [j, j+5) for all 128 tiles
    out_g = nc.declare_dram_parameter(
        "out_g", [NG, 16, WINB, NT, BW], OUT_DT, isOutput=True
    )

    with TileContext(nc) as tc:
        with (
            tc.tile_pool(name="bpool", bufs=1) as bpool,
            tc.tile_pool(name="apool", bufs=3) as apool,
            tc.tile_pool(name="spool", bufs=1) as spool,
            tc.tile_pool(name="psum", bufs=6, space="PSUM") as ppool,
        ):
            # whole padded in2 sample resident in SBUF (72.25KB/partition),
            # loaded in 4-row chunks so early tiles start before the full
            # 9.5MB lands (subtile deps give matmuls per-chunk waits)
            b_s = bpool.tile([128, KT, HP, WP], bf16)
            # all 128 tile-grams, block-interleaved: [part][blk][tile][32]
            st = spool.tile([128, NBLK, NT, BW], OUT_DT)

            def load_b(k):  # 4-row chunk k of the padded rows
                return nc.sync.dma_start(
                    out=b_s[:, :, 4 * k : 4 * k + 4],
                    in_=in2_p[:, :, 4 * k : 4 * k + 4],
                )

            a_tiles = {}

            def load_a(t):
                a_t = apool.tile([128, KT, NWT, 128], bf16, tag="a")
                a_tiles[t] = a_t
                return nc.sync.dma_start(out=a_t, in_=in1_t[:, t])

            # all input DMAs issued up front on the sync ring, interleaved
            # so stripe ht's in1 chunk lands beside the in2 rows it needs
            # (stripe ht reads padded rows [16ht, 16ht+24) = chunks < 4ht+6)
            ib = 0
            for ht in range(NHT):
                while ib < min(HP // 4, 4 * ht + 6):
                    load_b(ib)
                    ib += 1
                load_a(ht)
            while ib < HP // 4:
                load_b(ib)
                ib += 1

            for ht in range(NHT):
                r0 = TH * ht
                for wt in range(NWT):
                    w0 = TW * wt
                    idx = ht * NWT + wt
                    ps = ppool.tile([128, NR], mybir.dt.float32, name="ps", tag="ps")
                    for kt in range(KT):
                        nc.tensor.matmul(
                            ps,
                            a_tiles[ht][:, kt, wt, :],
                            b_s[:, kt, r0 : r0 + RH, w0 : w0 + RW],
                            start=(kt == 0),
                            stop=(kt == KT - 1),
                        )
                    dst = st[:, :, idx, :]
                    src = ps.rearrange("p (b w) -> p b w", w=BW)
                    if idx % 2 == 0:
                        nc.vector.tensor_copy(dst, src)
                    else:
                        nc.scalar.copy(dst, src)

            # 8 window writes (655KB each, 8KB-contiguous runs), alternating
            # HWDGE rings so each ring's ~2us HBM-write receipt hides behind
            # the other ring's data
            for j in range(NG):
                src = st[16 * j : 16 * j + 16, j : j + WINB, :, :]
                if j % 2 == 0:
                    nc.scalar.dma_start(out=out_g[j], in_=src)
                else:
                    nc.sync.dma_start(out=out_g[j], in_=src)

    nc.compile()
    return nc


def _get_nc():
    global _CACHED_NC
    if _CACHED_NC is None:
        _CACHED_NC = _build_nc()
    return _CACHED_NC


def _make_in_maps(in1: np.ndarray, in2: np.ndarray):
    in_maps = []
    for b in range(B):
        # [C,H,W] -> [c(128), ht, kt, wt, m=mh*8+mw]
        a = (
            in1[b]
            .astype(ml_dtypes.bfloat16)
            .reshape(KT, 128, NHT, TH, NWT, TW)
            .transpose(1, 2, 0, 4, 3, 5)
            .reshape(128, NHT, KT, NWT, 128)
        )
        p = np.zeros((KT, 128, HP, WP), ml_dtypes.bfloat16)
        p[:, :, D : D + H, D : D + W] = in2[b].astype(ml_dtypes.bfloat16).reshape(
            KT, 128, H, W
        )
        in_maps.append(
            {
                "in1_t": np.ascontiguousarray(a),
                "in2_p": np.ascontiguousarray(p.transpose(1, 0, 2, 3)),
            }
        )
    return in_maps


_M = np.arange(128)
_MH, _MW = _M >> 3, _M & 7
# window-local column of band entry (dy,dx) for partition-row m:
# n = (mh+dy)*16 + (mw+dx), window starts at 32*(mh>>1)
_QIDX = (
    16 * (_MH & 1)[:, None, None]
    + _MW[:, None, None]
    + 16 * np.arange(ND)[None, :, None]
    + np.arange(ND)[None, None, :]
).reshape(128, 1, ND * ND)


def _extract_band(g: np.ndarray) -> np.ndarray:
    """[NG, 16, WINB, NT, BW] window grams -> [81, H, W] cost volume."""
    win = (
        g.reshape(128, WINB, NT, BW)
        .transpose(0, 2, 1, 3)
        .reshape(128, NT, WINB * BW)
        .astype(np.float32)
    )
    band = np.take_along_axis(win, _QIDX, axis=2)  # [m, t, 81]
    # [m=(mh,mw), t=(ht,wt), d] -> [d, ht, mh, wt, mw] -> [81, H, W]
    band = band.reshape(TH, TW, NHT, NWT, ND * ND).transpose(4, 2, 0, 3, 1)
    return np.ascontiguousarray(band).reshape(ND * ND, H, W)


def kernel(**inputs) -> np.ndarray:
    in1 = np.ascontiguousarray(np.asarray(inputs["in1"], dtype=np.float32))
    in2 = np.ascontiguousarray(np.asarray(inputs["in2"], dtype=np.float32))
    assert in1.shape == (B, C, H, W) and in2.shape == (B, C, H, W)

    nc = _get_nc()
    in_maps = _make_in_maps(in1, in2)
    res = run_bass_kernel_spmd(nc, in_maps, list(range(B)))

    outs = [_extract_band(np.asarray(res.results[b]["out_g"])) for b in range(B)]
    return np.stack(outs).astype(np.float32)


# revision 3
# speedup vs baseline: 1.0913x; 1.0913x over previous
"""Correlation cost-volume kernel (max_displacement=4) for 8 Trainium2 cores.

Problem: in1, in2: [B=8, C=256, H=128, W=128] f32.
out[b, dy*9+dx, h, w] = sum_c in1[b,c,h,w] * pad(in2)[b, c, h+dy, w+dx]
(pad = 4 zeros on each spatial side), output [8, 81, 128, 128] f32.

Strategy (data-parallel, one batch sample per core):
  2D-tiled gram with tall tiles and partition-split output windows.  Each
  matmul tile covers 16h x 8w in1 pixels (M = 128 PSUM partitions, pixel
  (mh,mw) on partition m = 8*mh+mw) against its 24 x 16 halo of padded in2
  (N = 384 gram columns n = rh*16+rw, C contracted as two K=128 tiles).
  The 81-entry band for pixel (mh,mw) lives at n = (mh+dy)*16 + (mw+dx),
  i.e. inside [16*mh, 16*mh+144) -- so the LOWER 64 partitions (mh<8) only
  ever need columns [0,256) and the UPPER 64 (mh>=8) only [128,384).  Two
  partition-range PSUM->SBUF copies per tile store exactly those 256-column
  halves at identical byte offsets, giving a partition-uniform [128, 256]
  tile that ONE full-width (all 16 SDMA ports) DMA per 16-row stripe ships
  to HBM: 8.4 MB total vs 12.6 MB for the full gram.  The host gathers the
  per-pixel 81-band from the 256-column windows with numpy for free.

  DMA plumbing (the real bottlenecks found by tracing): HWDGE descriptor
  generation caps a stream of 1 KB-descriptor loads at ~180 GB/s, so in2
  ships in 16-row chunks (4.3 KB descriptors) and in1 in half-stripe chunks
  (4 KB descriptors).  Reads are split across BOTH HWDGE rings (in2 on
  sync, in1 on scalar) and each stripe's window write is issued as soon as
  its copies land, alternating rings, so the ~2us HBM-write receipts and
  the write data ride inside the read phase instead of draining after it.
"""

import ml_dtypes
import numpy as np

import concourse.bass as bass
import concourse.bacc as bacc
import concourse.mybir as mybir
from concourse.bass_utils import run_bass_kernel_spmd
from concourse.tile import TileContext

B, C, H, W = 8, 256, 128, 128
D = 4
ND = 2 * D + 1  # 9 displacements per axis
HP = H + 2 * D  # 136 padded rows
WP = W + 2 * D  # 136 padded cols
KT = C // 128  # 2 contraction tiles
TH, TW = 16, 8  # matmul tile = 16h x 8w pixels (128 = PSUM partition dim)
NHT, NWT = H // TH, W // TW  # 8 row-stripes x 16 tiles each
RH, RW = TH + 2 * D, TW + 2 * D  # 24 x 16 halo region
NR = RH * RW  # 384 gram columns per tile
NT = NHT * NWT  # 128 tiles
WC = 256  # stored window columns per pixel (half-split)
NAC = 2 * NHT  # 16 half-stripe in1 chunks

OUT_DT = mybir.dt.bfloat16
_OUT_NP = ml_dtypes.bfloat16

_CACHED_NC = None


def _build_nc():
    bf16 = mybir.dt.bfloat16

    nc = bacc.Bacc()
    # in1 as [c][chunk=2*ht+half][kt][w(8)][m=mh*8+mw]; in2 zero-padded in w
    # only, as [c][kt][h(128)][wp] -- the 8 pad rows are memset on-chip
    in1_t = nc.declare_dram_parameter("in1_t", [128, NAC, KT, 8, 128], bf16, isOutput=False)
    in2_p = nc.declare_dram_parameter("in2_p", [128, KT, H, WP], bf16, isOutput=False)
    # per-stripe window tiles: [ht][m][wt][256]
    out_c = nc.declare_dram_parameter(
        "out_c", [NHT, 128, NWT, WC], OUT_DT, isOutput=True
    )

    with TileContext(nc) as tc:
        with (
            tc.tile_pool(name="bpool", bufs=1) as bpool,
            tc.tile_pool(name="apool", bufs=5) as apool,
            tc.tile_pool(name="spool", bufs=1) as spool,
            tc.tile_pool(name="psum", bufs=6, space="PSUM") as ppool,
        ):
            # whole padded in2 sample resident in SBUF (72.25KB/partition)
            b_s = bpool.tile([128, KT, HP, WP], bf16)
            # zero the 4 pad rows top+bottom (pad cols ship from DRAM)
            nc.gpsimd.memset(b_s[:, :, 0:D, :], 0.0)
            nc.gpsimd.memset(b_s[:, :, D + H :, :], 0.0)
            # window store: [m][tile][256], partition-uniform (64KB/partition)
            st = spool.tile([128, NT, WC], OUT_DT)

            def load_b(k):  # 16-row chunk k -> 4.3KB descriptors
                return nc.sync.dma_start(
                    out=b_s[:, :, D + 16 * k : D + 16 * k + 16],
                    in_=in2_p[:, :, 16 * k : 16 * k + 16],
                )

            a_tiles = {}

            def load_a(c):  # half-stripe chunk -> 4KB descriptors
                a_t = apool.tile([128, KT, 8, 128], bf16, tag="a")
                a_tiles[c] = a_t
                return nc.scalar.dma_start(out=a_t, in_=in1_t[:, c])

            # reads split across both HWDGE rings: in2 on sync, in1 on
            # scalar, interleaved so stripe ht's inputs land proportionally
            ib = 0
            for ht in range(NHT):
                while ib < min(H // 16, ht + 2):
                    load_b(ib)
                    ib += 1
                load_a(2 * ht)
                load_a(2 * ht + 1)

            for ht in range(NHT):
                r0 = TH * ht
                for wt in range(NWT):
                    w0 = TW * wt
                    t = ht * NWT + wt
                    ps = ppool.tile([128, NR], mybir.dt.float32, name="ps", tag="ps")
                    ac = 2 * ht + wt // 8
                    for kt in range(KT):
                        nc.tensor.matmul(
                            ps,
                            a_tiles[ac][:, kt, wt % 8, :],
                            b_s[:, kt, r0 : r0 + RH, w0 : w0 + RW],
                            start=(kt == 0),
                            stop=(kt == KT - 1),
                        )
                    # partition-split window copies: lower pixel-rows keep
                    # gram cols [0,256), upper keep [128,384)
                    eng = nc.vector.tensor_copy if t % 2 == 0 else nc.scalar.copy
                    eng(st[0:64, t, :], ps[0:64, 0:WC])
                    eng(st[64:128, t, :], ps[64:128, NR - WC : NR])
                # ship the stripe as soon as its copies land; alternate
                # rings so receipts hide behind the other ring's traffic
                src = st[:, 16 * ht : 16 * ht + 16, :]
                if ht % 2 == 0:
                    nc.scalar.dma_start(out=out_c[ht], in_=src)
                else:
                    nc.sync.dma_start(out=out_c[ht], in_=src)

    nc.compile()
    return nc


def _get_nc():
    global _CACHED_NC
    if _CACHED_NC is None:
        _CACHED_NC = _build_nc()
    return _CACHED_NC


def _make_in_maps(in1: np.ndarray, in2: np.ndarray):
    in_maps = []
    for b in range(B):
        # [C,H,W] -> [c(128), chunk=2*ht+half, kt, w(8), m=mh*8+mw]
        a = (
            in1[b]
            .astype(ml_dtypes.bfloat16)
            .reshape(KT, 128, NHT, TH, 2, 8, TW)
            .transpose(1, 2, 4, 0, 5, 3, 6)
            .reshape(128, NAC, KT, 8, 128)
        )
        p = np.zeros((KT, 128, H, WP), ml_dtypes.bfloat16)
        p[:, :, :, D : D + W] = in2[b].astype(ml_dtypes.bfloat16).reshape(
            KT, 128, H, W
        )
        in_maps.append(
            {
                "in1_t": np.ascontiguousarray(a),
                "in2_p": np.ascontiguousarray(p.transpose(1, 0, 2, 3)),
            }
        )
    return in_maps


_M = np.arange(128)
_MH, _MW = _M >> 3, _M & 7
# window-local column of band entry (dy,dx) for partition m:
# n = (mh+dy)*16 + (mw+dx), window starts at 0 (mh<8) or 128 (mh>=8)
_QIDX = (
    16 * _MH[:, None, None]
    + _MW[:, None, None]
    + 16 * np.arange(ND)[None, :, None]
    + np.arange(ND)[None, None, :]
    - 128 * (_MH >= 8)[:, None, None]
).reshape(128, 1, ND * ND)


def _extract_band(g: np.ndarray) -> np.ndarray:
    """[NHT, 128, NWT, WC] window tiles -> [81, H, W] cost volume."""
    win = (
        g.reshape(NHT, 128, NWT, WC)
        .transpose(1, 0, 2, 3)
        .reshape(128, NT, WC)
        .astype(np.float32)
    )
    band = np.take_along_axis(win, _QIDX, axis=2)  # [m, t, 81]
    # [m=(mh,mw), t=(ht,wt), d] -> [d, ht, mh, wt, mw] -> [81, H, W]
    band = band.reshape(TH, TW, NHT, NWT, ND * ND).transpose(4, 2, 0, 3, 1)
    return np.ascontiguousarray(band).reshape(ND * ND, H, W)


def kernel(**inputs) -> np.ndarray:
    in1 = np.ascontiguousarray(np.asarray(inputs["in1"], dtype=np.float32))
    in2 = np.ascontiguousarray(np.asarray(inputs["in2"], dtype=np.float32))
    assert in1.shape == (B, C, H, W) and in2.shape == (B, C, H, W)

    nc = _get_nc()
    in_maps = _make_in_maps(in1, in2)
    res = run_bass_kernel_spmd(nc, in_maps, list(range(B)))

    outs = [_extract_band(np.asarray(res.results[b]["out_c"])) for b in range(B)]
    return np.stack(outs).astype(np.float32)


# revision 9
# speedup vs baseline: 1.1719x; 1.0738x over previous
"""Correlation cost-volume kernel (max_displacement=4) for 8 Trainium2 cores.

Problem: in1, in2: [B=8, C=256, H=128, W=128] f32.
out[b, dy*9+dx, h, w] = sum_c in1[b,c,h,w] * pad(in2)[b, c, h+dy, w+dx]
(pad = 4 zeros on each spatial side), output [8, 81, 128, 128] f32.

Strategy (data-parallel, one batch sample per core):
  2D-tiled gram with tall tiles and partition-split output windows.  Each
  matmul tile covers 16h x 8w in1 pixels (M = 128 PSUM partitions, pixel
  (mh,mw) on partition m = 8*mh+mw) against its 24 x 16 halo of padded in2
  (N = 384 gram columns n = rh*16+rw, C contracted as two K=128 tiles).
  The 81-entry band for pixel (mh,mw) lives at n = (mh+dy)*16 + (mw+dx),
  i.e. inside [16*mh, 16*mh+144) -- so the LOWER 64 partitions (mh<8) only
  ever need columns [0,256) and the UPPER 64 (mh>=8) only [128,384).  Two
  partition-range PSUM->SBUF copies per tile store exactly those 256-column
  halves at identical byte offsets, giving a partition-uniform [128, 256]
  tile that ONE full-width (all 16 SDMA ports) DMA per 16-row stripe ships
  to HBM: 8.4 MB total vs 12.6 MB for the full gram.  The host gathers the
  per-pixel 81-band from the 256-column windows with numpy for free.

  DMA plumbing (the real bottlenecks found by tracing): HWDGE descriptor
  generation caps a stream of 1 KB-descriptor loads at ~180 GB/s, so in2
  ships in 16-row chunks (4.3 KB descriptors) and in1 in half-stripe chunks
  (4 KB descriptors).  Reads are split across BOTH HWDGE rings (in2 on
  sync, in1 on scalar) and each stripe's window write is issued as soon as
  its copies land, alternating rings, so the ~2us HBM-write receipts and
  the write data ride inside the read phase instead of draining after it.
"""

import ml_dtypes
import numpy as np

import concourse.bass as bass
import concourse.bacc as bacc
import concourse.mybir as mybir
from concourse.bass_utils import run_bass_kernel_spmd
from concourse.tile import TileContext

B, C, H, W = 8, 256, 128, 128
D = 4
ND = 2 * D + 1  # 9 displacements per axis
HP = H + 2 * D  # 136 padded rows
WP = W + 2 * D  # 136 padded cols
KT = C // 128  # 2 contraction tiles
TH, TW = 16, 8  # matmul tile = 16h x 8w pixels (128 = PSUM partition dim)
NHT, NWT = H // TH, W // TW  # 8 row-stripes x 16 tiles each
RH, RW = TH + 2 * D, TW + 2 * D  # 24 x 16 halo region
NR = RH * RW  # 384 gram columns per tile
NT = NHT * NWT  # 128 tiles
WC = 256  # stored window columns per pixel (half-split)
NAC = 2 * NHT  # 16 half-stripe in1 chunks

OUT_DT = mybir.dt.bfloat16
_OUT_NP = ml_dtypes.bfloat16

_CACHED_NC = None


def _build_nc():
    bf16 = mybir.dt.bfloat16

    nc = bacc.Bacc()
    # in1 as [c][chunk=2*ht+half][kt][w(8)][m=mh*8+mw]; in2 zero-padded in w
    # only, as [c][kt][h(128)][wp] -- the 8 pad rows are memset on-chip
    in1_t = nc.declare_dram_parameter("in1_t", [128, NAC, KT, 8, 128], bf16, isOutput=False)
    # kt folded inside rows: one 8.7KB descriptor per partition per chunk
    in2_p = nc.declare_dram_parameter("in2_p", [128, H, KT, WP], bf16, isOutput=False)
    # per-stripe window tiles: [ht][m][wt][256]
    out_c = nc.declare_dram_parameter(
        "out_c", [NHT, 128, NWT, WC], OUT_DT, isOutput=True
    )

    with TileContext(nc) as tc:
        with (
            tc.tile_pool(name="bpool", bufs=1) as bpool,
            tc.tile_pool(name="apool", bufs=5) as apool,
            tc.tile_pool(name="spool", bufs=1) as spool,
            tc.tile_pool(name="psum", bufs=6, space="PSUM") as ppool,
        ):
            # whole padded in2 sample resident in SBUF (72.25KB/partition)
            b_s = bpool.tile([128, HP, KT, WP], bf16)
            # zero the 4 pad rows top+bottom (pad cols ship from DRAM)
            nc.gpsimd.memset(b_s[:, 0:D], 0.0)
            nc.gpsimd.memset(b_s[:, D + H :], 0.0)
            # full tile-grams: [m][tile][384] (96KB/partition)
            st = spool.tile([128, NT, NR], OUT_DT)

            def load_b(k):  # 16-row chunk k -> 8.7KB descriptors
                return nc.sync.dma_start(
                    out=b_s[:, D + 16 * k : D + 16 * k + 16],
                    in_=in2_p[:, 16 * k : 16 * k + 16],
                )

            a_tiles = {}

            def load_a(c):  # half-stripe chunk -> 4KB descriptors
                a_t = apool.tile([128, KT, 8, 128], bf16, tag="a")
                a_tiles[c] = a_t
                return nc.scalar.dma_start(out=a_t, in_=in1_t[:, c])

            # reads split across both HWDGE rings: in2 on sync, in1 on
            # scalar, interleaved so stripe ht's inputs land proportionally
            ib = 0
            for ht in range(NHT):
                while ib < min(H // 16, ht + 2):
                    load_b(ib)
                    ib += 1
                load_a(2 * ht)
                load_a(2 * ht + 1)

            for ht in range(NHT):
                r0 = TH * ht
                for wt in range(NWT):
                    w0 = TW * wt
                    t = ht * NWT + wt
                    ps = ppool.tile([128, NR], mybir.dt.float32, name="ps", tag="ps")
                    ac = 2 * ht + wt // 8
                    for kt in range(KT):
                        nc.tensor.matmul(
                            ps,
                            a_tiles[ac][:, kt, wt % 8, :],
                            b_s[:, r0 : r0 + RH, kt, w0 : w0 + RW],
                            start=(kt == 0),
                            stop=(kt == KT - 1),
                        )
                    # one cheap full-gram copy per tile (PSUM drain on the
                    # two PSUM-capable engines is the pipeline's scarce
                    # resource: 384 elems fits the stripe cadence, 512 not)
                    if t % 2 == 0:
                        nc.vector.tensor_copy(st[:, t, :], ps)
                    else:
                        nc.scalar.copy(st[:, t, :], ps)
                # window selection happens in the WRITE: lower pixel-rows
                # (mh<8, partitions 0:64 = the 8 even SBUF ports) only need
                # gram cols [0,256); upper rows (odd ports) only [128,384).
                # The lo/hi pair rides disjoint port halves on different
                # rings, so each stripe ships 2x256 cols at combined full
                # width as soon as its copies land.
                ts = slice(16 * ht, 16 * ht + 16)
                nc.scalar.dma_start(out=out_c[ht, 0:64], in_=st[0:64, ts, 0:WC])
                nc.sync.dma_start(out=out_c[ht, 64:128], in_=st[64:128, ts, NR - WC : NR])

    nc.compile()
    return nc


def _get_nc():
    global _CACHED_NC
    if _CACHED_NC is None:
        _CACHED_NC = _build_nc()
    return _CACHED_NC


def _make_in_maps(in1: np.ndarray, in2: np.ndarray):
    in_maps = []
    for b in range(B):
        # [C,H,W] -> [c(128), chunk=2*ht+half, kt, w(8), m=mh*8+mw]
        a = (
            in1[b]
            .astype(ml_dtypes.bfloat16)
            .reshape(KT, 128, NHT, TH, 2, 8, TW)
            .transpose(1, 2, 4, 0, 5, 3, 6)
            .reshape(128, NAC, KT, 8, 128)
        )
        p = np.zeros((KT, 128, H, WP), ml_dtypes.bfloat16)
        p[:, :, :, D : D + W] = in2[b].astype(ml_dtypes.bfloat16).reshape(
            KT, 128, H, W
        )
        in_maps.append(
            {
                "in1_t": np.ascontiguousarray(a),
                # [c, h, kt, wp]
                "in2_p": np.ascontiguousarray(p.transpose(1, 2, 0, 3)),
            }
        )
    return in_maps


_M = np.arange(128)
_MH, _MW = _M >> 3, _M & 7
# window-local column of band entry (dy,dx) for partition m:
# n = (mh+dy)*16 + (mw+dx), window starts at 0 (mh<8) or 128 (mh>=8)
_QIDX = (
    16 * _MH[:, None, None]
    + _MW[:, None, None]
    + 16 * np.arange(ND)[None, :, None]
    + np.arange(ND)[None, None, :]
    - 128 * (_MH >= 8)[:, None, None]
).reshape(128, 1, ND * ND)


def _extract_band(g: np.ndarray) -> np.ndarray:
    """[NHT, 128, NWT, WC] window tiles -> [81, H, W] cost volume."""
    win = (
        g.reshape(NHT, 128, NWT, WC)
        .transpose(1, 0, 2, 3)
        .reshape(128, NT, WC)
        .astype(np.float32)
    )
    band = np.take_along_axis(win, _QIDX, axis=2)  # [m, t, 81]
    # [m=(mh,mw), t=(ht,wt), d] -> [d, ht, mh, wt, mw] -> [81, H, W]
    band = band.reshape(TH, TW, NHT, NWT, ND * ND).transpose(4, 2, 0, 3, 1)
    return np.ascontiguousarray(band).reshape(ND * ND, H, W)


def kernel(**inputs) -> np.ndarray:
    in1 = np.ascontiguousarray(np.asarray(inputs["in1"], dtype=np.float32))
    in2 = np.ascontiguousarray(np.asarray(inputs["in2"], dtype=np.float32))
    assert in1.shape == (B, C, H, W) and in2.shape == (B, C, H, W)

    nc = _get_nc()
    in_maps = _make_in_maps(in1, in2)
    res = run_bass_kernel_spmd(nc, in_maps, list(range(B)))

    outs = [_extract_band(np.asarray(res.results[b]["out_c"])) for b in range(B)]
    return np.stack(outs).astype(np.float32)


# revision 13
# speedup vs baseline: 1.2010x; 1.0249x over previous
"""Correlation cost-volume kernel (max_displacement=4) for 8 Trainium2 cores.

Problem: in1, in2: [B=8, C=256, H=128, W=128] f32.
out[b, dy*9+dx, h, w] = sum_c in1[b,c,h,w] * pad(in2)[b, c, h+dy, w+dx]
(pad = 4 zeros on each spatial side), output [8, 81, 128, 128] f32.

Strategy (data-parallel, one batch sample per core):
  2D-tiled gram with tall tiles and partition-split output windows.  Each
  matmul tile covers 16h x 8w in1 pixels (M = 128 PSUM partitions, pixel
  (mh,mw) on partition m = 8*mh+mw) against its 24 x 16 halo of padded in2
  (N = 384 gram columns n = rh*16+rw, C contracted as two K=128 tiles).
  The 81-entry band for pixel (mh,mw) lives at n = (mh+dy)*16 + (mw+dx),
  i.e. inside [16*mh, 16*mh+144) -- so the LOWER 64 partitions (mh<8) only
  ever need columns [0,256) and the UPPER 64 (mh>=8) only [128,384).  Two
  partition-range PSUM->SBUF copies per tile store exactly those 256-column
  halves at identical byte offsets, giving a partition-uniform [128, 256]
  tile that ONE full-width (all 16 SDMA ports) DMA per 16-row stripe ships
  to HBM: 8.4 MB total vs 12.6 MB for the full gram.  The host gathers the
  per-pixel 81-band from the 256-column windows with numpy for free.

  DMA plumbing (the real bottlenecks found by tracing): HWDGE descriptor
  generation caps a stream of 1 KB-descriptor loads at ~180 GB/s, so in2
  ships in 16-row chunks (4.3 KB descriptors) and in1 in half-stripe chunks
  (4 KB descriptors).  Reads are split across BOTH HWDGE rings (in2 on
  sync, in1 on scalar) and each stripe's window write is issued as soon as
  its copies land, alternating rings, so the ~2us HBM-write receipts and
  the write data ride inside the read phase instead of draining after it.
"""

import ml_dtypes
import numpy as np

import concourse.bass as bass
import concourse.bacc as bacc
import concourse.mybir as mybir
from concourse.bass_utils import run_bass_kernel_spmd
from concourse.tile import TileContext

B, C, H, W = 8, 256, 128, 128
D = 4
ND = 2 * D + 1  # 9 displacements per axis
HP = H + 2 * D  # 136 padded rows
WP = W + 2 * D  # 136 padded cols
KT = C // 128  # 2 contraction tiles
TH, TW = 16, 8  # matmul tile = 16h x 8w pixels (128 = PSUM partition dim)
NHT, NWT = H // TH, W // TW  # 8 row-stripes x 16 tiles each
RH, RW = TH + 2 * D, TW + 2 * D  # 24 x 16 halo region
NR = RH * RW  # 384 gram columns per tile
NT = NHT * NWT  # 128 tiles
WC = 256  # stored window columns per pixel (half-split)
NAC = 2 * NHT  # 16 half-stripe in1 chunks

OUT_DT = mybir.dt.bfloat16
_OUT_NP = ml_dtypes.bfloat16

_CACHED_NC = None


def _build_nc():
    bf16 = mybir.dt.bfloat16

    nc = bacc.Bacc()
    # in1 as [c][chunk=2*ht+half][kt][w(8)][m=mh*8+mw]; in2 zero-padded in w
    # only, as [c][kt][h(128)][wp] -- the 8 pad rows are memset on-chip
    in1_t = nc.declare_dram_parameter("in1_t", [128, NAC, KT, 8, 128], bf16, isOutput=False)
    # kt folded inside rows: one 8.7KB descriptor per partition per chunk
    in2_p = nc.declare_dram_parameter("in2_p", [128, H, KT, WP], bf16, isOutput=False)
    # per-stripe tile-grams: [ht][m][wt][384]
    out_c = nc.declare_dram_parameter(
        "out_c", [NHT, 128, NWT, NR], OUT_DT, isOutput=True
    )

    with TileContext(nc) as tc:
        with (
            tc.tile_pool(name="bpool", bufs=1) as bpool,
            tc.tile_pool(name="apool", bufs=5) as apool,
            tc.tile_pool(name="spool", bufs=1) as spool,
            tc.tile_pool(name="psum", bufs=8, space="PSUM") as ppool,
        ):
            # whole padded in2 sample resident in SBUF (72.25KB/partition)
            b_s = bpool.tile([128, HP, KT, WP], bf16)
            # zero the 4 pad rows top+bottom (pad cols ship from DRAM)
            nc.gpsimd.memset(b_s[:, 0:D], 0.0)
            nc.gpsimd.memset(b_s[:, D + H :], 0.0)
            # full tile-grams: [m][tile][384] (96KB/partition)
            st = spool.tile([128, NT, NR], OUT_DT)

            def load_b(k):  # 16-row chunk k -> 8.7KB descriptors
                return nc.sync.dma_start(
                    out=b_s[:, D + 16 * k : D + 16 * k + 16],
                    in_=in2_p[:, 16 * k : 16 * k + 16],
                )

            a_tiles = {}

            def load_a(c):  # half-stripe chunk -> 4KB descriptors
                a_t = apool.tile([128, KT, 8, 128], bf16, tag="a")
                a_tiles[c] = a_t
                return nc.scalar.dma_start(out=a_t, in_=in1_t[:, c])

            # reads split across both HWDGE rings: in2 on sync, in1 on
            # scalar, interleaved so stripe ht's inputs land proportionally
            ib = 0
            for ht in range(NHT):
                while ib < min(H // 16, ht + 2):
                    load_b(ib)
                    ib += 1
                load_a(2 * ht)
                load_a(2 * ht + 1)

            for ht in range(NHT):
                r0 = TH * ht
                for wt in range(NWT):
                    w0 = TW * wt
                    t = ht * NWT + wt
                    ps = ppool.tile([128, NR], mybir.dt.float32, name="ps", tag="ps")
                    ac = 2 * ht + wt // 8
                    for kt in range(KT):
                        nc.tensor.matmul(
                            ps,
                            a_tiles[ac][:, kt, wt % 8, :],
                            b_s[:, r0 : r0 + RH, kt, w0 : w0 + RW],
                            start=(kt == 0),
                            stop=(kt == KT - 1),
                        )
                    # one cheap full-gram copy per tile (PSUM drain on the
                    # two PSUM-capable engines is the pipeline's scarce
                    # resource: 384 elems fits the stripe cadence, 512 not)
                    if t % 2 == 0:
                        nc.vector.tensor_copy(st[:, t, :], ps)
                    else:
                        nc.scalar.copy(st[:, t, :], ps)
                # ship the whole stripe-gram as one full-width DMA (128
                # descriptors of 12KB -- descgen stays off the copy engines'
                # critical path) on the otherwise-idle sync ring; each write
                # fires as soon as its stripe's copies land, so the 12.6MB
                # of writes ride the read/compute phase
                nc.sync.dma_start(
                    out=out_c[ht], in_=st[:, 16 * ht : 16 * ht + 16, :]
                )

    nc.compile()
    return nc


def _get_nc():
    global _CACHED_NC
    if _CACHED_NC is None:
        _CACHED_NC = _build_nc()
    return _CACHED_NC


def _make_in_maps(in1: np.ndarray, in2: np.ndarray):
    in_maps = []
    for b in range(B):
        # [C,H,W] -> [c(128), chunk=2*ht+half, kt, w(8), m=mh*8+mw]
        a = (
            in1[b]
            .astype(ml_dtypes.bfloat16)
            .reshape(KT, 128, NHT, TH, 2, 8, TW)
            .transpose(1, 2, 4, 0, 5, 3, 6)
            .reshape(128, NAC, KT, 8, 128)
        )
        p = np.zeros((KT, 128, H, WP), ml_dtypes.bfloat16)
        p[:, :, :, D : D + W] = in2[b].astype(ml_dtypes.bfloat16).reshape(
            KT, 128, H, W
        )
        in_maps.append(
            {
                "in1_t": np.ascontiguousarray(a),
                # [c, h, kt, wp]
                "in2_p": np.ascontiguousarray(p.transpose(1, 2, 0, 3)),
            }
        )
    return in_maps


_M = np.arange(128)
_MH, _MW = _M >> 3, _M & 7
# gram column of band entry (dy,dx) for partition m: n = (mh+dy)*16 + (mw+dx)
_QIDX = (
    16 * _MH[:, None, None]
    + _MW[:, None, None]
    + 16 * np.arange(ND)[None, :, None]
    + np.arange(ND)[None, None, :]
).reshape(128, 1, ND * ND)


def _extract_band(g: np.ndarray) -> np.ndarray:
    """[NHT, 128, NWT, NR] stripe tile-grams -> [81, H, W] cost volume."""
    win = (
        g.reshape(NHT, 128, NWT, NR)
        .transpose(1, 0, 2, 3)
        .reshape(128, NT, NR)
        .astype(np.float32)
    )
    band = np.take_along_axis(win, _QIDX, axis=2)  # [m, t, 81]
    # [m=(mh,mw), t=(ht,wt), d] -> [d, ht, mh, wt, mw] -> [81, H, W]
    band = band.reshape(TH, TW, NHT, NWT, ND * ND).transpose(4, 2, 0, 3, 1)
    return np.ascontiguousarray(band).reshape(ND * ND, H, W)


def kernel(**inputs) -> np.ndarray:
    in1 = np.ascontiguousarray(np.asarray(inputs["in1"], dtype=np.float32))
    in2 = np.ascontiguousarray(np.asarray(inputs["in2"], dtype=np.float32))
    assert in1.shape == (B, C, H, W) and in2.shape == (B, C, H, W)

    nc = _get_nc()
    in_maps = _make_in_maps(in1, in2)
    res = run_bass_kernel_spmd(nc, in_maps, list(range(B)))

    outs = [_extract_band(np.asarray(res.results[b]["out_c"])) for b in range(B)]
    return np.stack(outs).astype(np.float32)
